# revision 1
# baseline (speedup 1.0000x reference)
"""GQA forward kernel for Trainium2, 8-core tensor-parallel (group-aligned).

Problem: B=2, T=2048, D=2048, 32 Q heads / 8 KV heads, head_dim 64, causal.

Sharding: core c owns KV head c and its 4 Q heads (whole GQA group), both
batches.  Output projection is row-parallel Megatron style: each core
contracts its 256 attention-output channels against its slice of Wo and the
host sums the 8 partial outputs (+ bo).

Device-side dataflow per core (matmuls in float32r unless noted, fp32 accum):
  x^T [C, T] (host-transposed)
    -> QKK proj:  lhsT = [Wq_c | Wk_c | Wk_c]  -> Q^T [256, T], K^T dup [128, T]
    -> V proj (fp16): lhsT = x^T fp16, rhs = Wv_c fp16 -> V [T, 64] natural
  attention per (batch, head-pair, q-chunk of 512), scores TRANSPOSED:
    S^T[kv, q] = matmul(lhsT=K^T tile [64,128], rhs=Q^T [64, 512])
      head pairs run on disjoint PE row groups (base partitions 0 / 64)
    expS = ACT Exp(S^T / 8)  (no max-subtraction: |scores| <= ~6)
    causal: column-sliced matmuls + one triangle mask on diagonal tiles
    AV: matmul(lhsT=V2 [kv,65] (V plus ones col), rhs=expS) accumulated over
        kv tiles -> [attn^T; den] in PSUM
    normalize: den replicated to 64 partitions via K=1 ones-matmul,
        reciprocal + multiply on DVE
  out-proj: y[t, e] = matmul(lhsT=attn^T [256, t], rhs=Wo_c [256, e])
"""

import os

import numpy as np

import concourse.mybir as mybir
import concourse.tile as tile
from concourse import bacc
from concourse import bass_utils

P = 128
B = 2
T = 2048
C = 2048
HD = 64
QH = 32
KVH = 8
G = QH // KVH  # 4
NCORES = 8
QH_LOC = QH // NCORES  # 4 q heads per core
TCH = 256  # token chunk for projection phase
QCH = 512  # q chunk for attention phase
KT = C // P  # 16 contraction tiles
f32 = mybir.dt.float32
f32r = mybir.dt.float32r
bf16 = mybir.dt.bfloat16
fp16 = mybir.dt.float16

_CACHE = {}


def _build():
    nc = bacc.Bacc("TRN2", target_bir_lowering=False, debug=False, num_devices=NCORES)

    xt = nc.dram_tensor("xt", [B, C, T], f32, kind="ExternalInput")
    xtb = nc.dram_tensor("xtb", [B, C, T], fp16, kind="ExternalInput")
    wqk = nc.dram_tensor("wqk", [C, 384], f32, kind="ExternalInput")
    wv = nc.dram_tensor("wv", [C, HD], fp16, kind="ExternalInput")
    wo = nc.dram_tensor("wo", [G * HD, C], f32, kind="ExternalInput")
    bqk = nc.dram_tensor("bqk", [P, 3], f32, kind="ExternalInput")
    bv = nc.dram_tensor("bv", [1, HD], f32, kind="ExternalInput")
    maskd = nc.dram_tensor("mask", [P, P], f32, kind="ExternalInput")
    y = nc.dram_tensor("y", [B, T, C], f32, kind="ExternalOutput")

    wqk3 = wqk.ap().rearrange("(ko p) m -> p ko m", p=P).bitcast(f32r)
    wv3 = wv.ap().rearrange("(ko p) m -> p ko m", p=P)
    wo3 = wo.ap().rearrange("(ko p) m -> p ko m", p=P).bitcast(f32r)

    with tile.TileContext(nc) as tc:
        with (
            tc.tile_pool(name="const", bufs=1) as cpool,
            tc.tile_pool(name="x", bufs=2) as xpool,
            tc.tile_pool(name="proj", bufs=1) as projpool,
            tc.tile_pool(name="attn", bufs=1) as apool,
            tc.tile_pool(name="work", bufs=5) as wpool,
            tc.tile_pool(name="work2", bufs=6) as wpool2,
            tc.tile_pool(name="psA", bufs=2, space="PSUM") as psumA,
            tc.tile_pool(name="psB", bufs=2, space="PSUM") as psumB,
            tc.tile_pool(name="psC", bufs=2, space="PSUM") as psumC,
        ):
            # ---- constants / weights (resident) ----
            # startup-critical DMA order: wqk sub0, then x chunk 0 (the first
            # 16 QKK matmuls need only these), then the rest
            wqk_sb = cpool.tile([P, KT, 384], f32r)
            nc.sync.dma_start(wqk_sb[:, :, 0:P], wqk3[:, :, 0:P])
            xb0 = xt.ap()[0].rearrange("(ko p) t -> p ko t", p=P).bitcast(f32r)
            xbb0 = xtb.ap()[0].rearrange("(ko p) t -> p ko t", p=P)
            xch0 = xpool.tile([P, KT, TCH], f32r, tag="xch", name="xch")
            nc.sync.dma_start(xch0[:, 0 : KT // 2, :], xb0[:, 0 : KT // 2, 0:TCH])
            nc.sync.dma_start(xch0[:, KT // 2 :, :], xb0[:, KT // 2 :, 0:TCH])
            for _s in range(1, 3):
                nc.sync.dma_start(
                    wqk_sb[:, :, _s * P : (_s + 1) * P], wqk3[:, :, _s * P : (_s + 1) * P]
                )
            xchb0 = xpool.tile([P, KT, TCH], fp16, tag="xchb", name="xchb")
            nc.sync.dma_start(xchb0[:, 0 : KT // 2, :], xbb0[:, 0 : KT // 2, 0:TCH])
            nc.sync.dma_start(xchb0[:, KT // 2 :, :], xbb0[:, KT // 2 :, 0:TCH])
            wv_sb = cpool.tile([P, KT, HD], fp16)
            nc.sync.dma_start(wv_sb[:], wv3)
            bqk_sb = cpool.tile([P, 3], f32)
            nc.sync.dma_start(bqk_sb[:], bqk.ap())
            bv_sb = cpool.tile([P, HD], f32)
            nc.sync.dma_start(bv_sb[:], bv.ap().to_broadcast((P, HD)))
            mask_sb = cpool.tile([P, P], f32r)
            nc.sync.dma_start(mask_sb[:], maskd.ap().bitcast(f32r))
            ones_f32 = cpool.tile([P, KT], f32)
            nc.gpsimd.memset(ones_f32[:], 1.0)
            ones_r = cpool.tile([P, HD], f32r)
            nc.vector.tensor_copy(ones_r[:], ones_f32[:, 0:1].to_broadcast((P, HD)))
            wo_sb = cpool.tile([P, 2, C], f32r)

            def emit_p3(pb, pattn, pqc):
                for ts in range(pqc * (QCH // P), (pqc + 1) * (QCH // P)):
                    for ec in range(C // QCH):
                        py = psumC.tile([P, QCH], f32, tag="pp", name="py")
                        for ks in range(2):
                            nc.tensor.matmul(
                                py[:],
                                pattn[:, ks, ts * P : (ts + 1) * P],
                                wo_sb[:, ks, ec * QCH : (ec + 1) * QCH],
                                start=(ks == 0),
                                stop=(ks == 1),
                            )
                        y_sb = wpool2.tile([P, QCH], f32, tag="ysb")
                        nc.any.tensor_copy(y_sb[:], py[:])
                        nc.sync.dma_start(
                            y.ap()[
                                pb, ts * P : (ts + 1) * P, ec * QCH : (ec + 1) * QCH
                            ],
                            y_sb[:],
                        )

            deferred_p3 = None
            for b in range(B):
                xb = xt.ap()[b].rearrange("(ko p) t -> p ko t", p=P).bitcast(f32r)
                xbb = xtb.ap()[b].rearrange("(ko p) t -> p ko t", p=P)

                # ---- P1: projections ----
                qkk_sb = projpool.tile([P, 3, T], f32r, tag="qkk")
                v2_sb = projpool.tile([P, KT, 130], f32r, tag="v2")
                nc.vector.tensor_copy(v2_sb[:, :, 64:65], ones_f32[:, :, None])
                for tch in range(T // TCH):
                    tsl = slice(tch * TCH, (tch + 1) * TCH)
                    if b == 0 and tch == 0:
                        xch, xchb = xch0, xchb0
                    else:
                        xch = xpool.tile([P, KT, TCH], f32r, tag="xch", name="xch")
                        nc.sync.dma_start(xch[:, 0 : KT // 2, :], xb[:, 0 : KT // 2, tsl])
                        nc.sync.dma_start(xch[:, KT // 2 :, :], xb[:, KT // 2 :, tsl])
                        xchb = xpool.tile([P, KT, TCH], fp16, tag="xchb", name="xchb")
                        nc.sync.dma_start(xchb[:, 0 : KT // 2, :], xbb[:, 0 : KT // 2, tsl])
                        nc.sync.dma_start(xchb[:, KT // 2 :, :], xbb[:, KT // 2 :, tsl])
                    if tch == 4 and b == 0:
                        nc.sync.dma_start(wo_sb[:], wo3)
                    if tch == 3 and deferred_p3 is not None:
                        emit_p3(*deferred_p3)
                        deferred_p3 = None
                    for sub in range(3):
                        pp_full = psumC.tile([P, QCH], f32, tag="pp", name="pp")
                        pp = pp_full[:, :TCH]
                        for k in range(KT):
                            nc.tensor.matmul(
                                pp[:],
                                wqk_sb[:, k, sub * P : (sub + 1) * P],
                                xch[:, k, :],
                                start=(k == 0),
                                stop=(k == KT - 1),
                            )
                        nc.any.tensor_tensor(
                            qkk_sb[:, sub, tsl],
                            pp[:],
                            bqk_sb[:, sub : sub + 1].to_broadcast((P, TCH)),
                            mybir.AluOpType.add,
                        )
                    for ts in range(TCH // P):
                        tidx = tch * (TCH // P) + ts
                        pv = psumC.tile([P, HD], f32, tag="pp", name="pv")
                        for k in range(KT):
                            nc.tensor.matmul(
                                pv[:],
                                xchb[:, k, ts * P : (ts + 1) * P],
                                wv_sb[:, k, :],
                                start=(k == 0),
                                stop=(k == KT - 1),
                            )
                        nc.any.tensor_tensor(
                            v2_sb[:, tidx, 0:64], pv[:], bv_sb[:], mybir.AluOpType.add
                        )
                        nc.any.tensor_tensor(
                            v2_sb[:, tidx, 65:129], pv[:], bv_sb[:], mybir.AluOpType.add
                        )

                # ---- P2 + P3 interleaved: attention then out-proj per q-chunk ----
                # Head pairs (2*sub, 2*sub+1) run QK^T on disjoint PE row
                # groups (base partitions 0 / 64); their score tiles share one
                # 2-bank PSUM tile so exp is a single wide ACT op.
                attn_sb = apool.tile([P, 2, T], f32r, tag="attn")
                for qc in range(T // QCH):
                    q0 = qc * QCH
                    nfull = q0 // P
                    ntiles = nfull + QCH // P
                    for sub in range(2):
                        qT0 = qkk_sb[0:64, sub, q0 : q0 + QCH]
                        qT1 = qkk_sb[64:128, sub, q0 : q0 + QCH]
                        pav0 = psumB.tile([P, QCH], f32, tag="pav", name="pav0")
                        pav1 = psumB.tile([P, QCH], f32, tag="pav", name="pav1")
                        for i in range(ntiles):
                            if i < nfull:
                                nsl = slice(0, QCH)
                            else:
                                nsl = slice((i - nfull) * P, QCH)
                            ksl = slice(i * P, (i + 1) * P)
                            ps_s = psumA.tile([P, 2, QCH], f32, tag="ps", name="ps_s")
                            # concurrent pair: disjoint PE row groups 0-63 / 64-127
                            nc.tensor.matmul(
                                ps_s[:, 0, nsl],
                                qkk_sb[0:64, 2, ksl],
                                qT0[:, nsl],
                                start=True,
                                stop=True,
                            )
                            nc.tensor.matmul(
                                ps_s[:, 1, nsl],
                                qkk_sb[64:128, 2, ksl],
                                qT1[:, nsl],
                                start=True,
                                stop=True,
                            )
                            expS = wpool.tile([P, 2, QCH], f32r, tag="expS")
                            nc.scalar.activation(
                                expS[:, :, nsl],
                                ps_s[:, :, nsl],
                                mybir.ActivationFunctionType.Exp,
                                scale=0.125,
                            )
                            if i >= nfull:
                                j = i - nfull
                                nc.any.tensor_tensor(
                                    expS[:, :, j * P : (j + 1) * P],
                                    expS[:, :, j * P : (j + 1) * P],
                                    mask_sb[:, None, :].to_broadcast((P, 2, P)),
                                    mybir.AluOpType.mult,
                                )
                            for half, pav in ((0, pav0), (1, pav1)):
                                nc.tensor.matmul(
                                    pav[0:65, nsl],
                                    v2_sb[:, i, 0:65],
                                    expS[:, half, nsl],
                                    start=(i == 0),
                                    stop=(i == ntiles - 1),
                                    skip_group_check=True,
                                )
                        for half, pav in ((0, pav0), (1, pav1)):
                            den_sb = wpool2.tile([P, QCH], f32r, tag="den")
                            nc.any.tensor_copy(den_sb[64:65, :], pav[64:65, :])
                            ps_den = psumA.tile([64, QCH], f32, tag="ps", name="psd")
                            nc.tensor.matmul(
                                ps_den[:],
                                ones_r[64:65, 0:64],
                                den_sb[64:65, :],
                                start=True,
                                stop=True,
                            )
                            rec = wpool2.tile([64, QCH], f32, tag="rec")
                            nc.vector.reciprocal(rec[:], ps_den[:])
                            if half == 0:
                                nc.any.tensor_tensor(
                                    attn_sb[0:64, sub, q0 : q0 + QCH],
                                    pav[0:64, :],
                                    rec[:],
                                    mybir.AluOpType.mult,
                                )
                            else:
                                alo = wpool2.tile([64, QCH], f32r, tag="alo")
                                nc.any.tensor_tensor(
                                    alo[:], pav[0:64, :], rec[:], mybir.AluOpType.mult
                                )
                                nc.sync.dma_start(
                                    attn_sb[64:128, sub, q0 : q0 + QCH], alo[:]
                                )

                    # out-proj for the finished token range; the last q-chunk is
                    # deferred into the next batch's P1 (fills PE during DMA waits)
                    if qc < T // QCH - 1 or b == B - 1:
                        emit_p3(b, attn_sb, qc)
                    else:
                        deferred_p3 = (b, attn_sb, qc)

            if deferred_p3 is not None:
                emit_p3(*deferred_p3)

    nc.compile()
    return nc


def _prep_inputs(x, Wq, bq, Wk, bk, Wv, bv, Wo, bo):
    x = np.ascontiguousarray(np.asarray(x, dtype=np.float32))
    xt = np.ascontiguousarray(x.transpose(0, 2, 1))
    xtb = xt.astype(np.float16)
    Wq = np.asarray(Wq, dtype=np.float32)
    Wk = np.asarray(Wk, dtype=np.float32)
    Wv = np.asarray(Wv, dtype=np.float32)
    Wo = np.asarray(Wo, dtype=np.float32)
    bq = np.asarray(bq, dtype=np.float32)
    bk = np.asarray(bk, dtype=np.float32)
    bv = np.asarray(bv, dtype=np.float32)

    # mask[kj, qi] = 1 iff kj <= qi  (upper triangular incl. diag)
    mask = np.triu(np.ones((P, P), dtype=np.float32)).copy()
    in_maps = []
    for c in range(NCORES):
        qs = slice(c * G * HD, (c + 1) * G * HD)
        ks = slice(c * HD, (c + 1) * HD)
        wqk_c = np.concatenate([Wq[:, qs], Wk[:, ks], Wk[:, ks]], axis=1)
        bq_c = bq[qs]
        bqk_c = np.stack(
            [bq_c[0:128], bq_c[128:256], np.concatenate([bk[ks], bk[ks]])], axis=1
        )
        in_maps.append(
            {
                "xt": xt,
                "xtb": xtb,
                "wqk": np.ascontiguousarray(wqk_c),
                "wv": np.ascontiguousarray(Wv[:, ks]).astype(np.float16),
                "wo": np.ascontiguousarray(Wo[qs, :]),
                "bqk": np.ascontiguousarray(bqk_c),
                "bv": np.ascontiguousarray(bv[None, ks]),
                "mask": mask,
            }
        )
    return in_maps


def kernel(x, Wq, bq, Wk, bk, Wv, bv, Wo, bo, _trace=False):
    # NTFF tracing is unavailable through this axon client; make sure a
    # stray BASS_TRACE=1 in the environment cannot divert the run path.
    if not _trace:
        os.environ["BASS_NEVER_TRACE"] = "1"
    if "nc" not in _CACHE:
        _CACHE["nc"] = _build()
    nc = _CACHE["nc"]
    in_maps = _prep_inputs(x, Wq, bq, Wk, bk, Wv, bv, Wo, bo)
    res = bass_utils.run_bass_kernel_spmd(
        nc, in_maps, core_ids=list(range(NCORES)), trace=_trace
    )
    bo = np.asarray(bo, dtype=np.float32)
    y = np.zeros((B, T, C), dtype=np.float32)
    for c in range(NCORES):
        y += res.results[c]["y"]
    y += bo
    if _trace:
        return y, res
    return y



# revision 28
# speedup vs baseline: 1.2177x; 1.2177x over previous
"""GQA forward kernel for Trainium2, 8-core tensor-parallel (group-aligned).

Problem: B=2, T=2048, D=2048, 32 Q heads / 8 KV heads, head_dim 64, causal.

Sharding: core c = (batch b = c//4, kv-head pair j = c%4).  Each core owns
kv heads {2j, 2j+1} and their 8 q heads for ONE batch.  Each core reads only
its batch's x^T (fp16) and emits a row-parallel partial of the output
projection (fp16); the host sums 4 partials per batch (+ bo).

All matmuls in fp16 with fp32 PSUM accumulation (tolerance is 2e-2; fp16
keeps rel err ~1e-3 and always hits 1.0 cycles/row on the PE cost model).

Per-core dataflow:
  P1 (proj, 4 chunks of 512 tokens): lhsT = wqkv [C,768] fp16, rhs = x^T
    chunk -> 6 sub-blocks of 128: subs 0-3 = Q pairs [g0hi | g1hi],
    sub 4 = K2 = [K_g0 | K_g1]^T, sub 5 = V2^T (transposed to natural via
    PE identity-matmul transposes).  v2 layout [kv, 129] = [V_g0 | 1 | V_g1].
  P2 attention per (pair i, q-chunk of 512): scores transposed,
    S^T[kv, q] for both heads of the pair in one PSUM tile [128,2,512]
    (head g0hi contracts K2[0:64], g1hi contracts K2[64:128]).
    expS = ACT Exp(S/8) -> fp16; causal via column-sliced matmuls + one
    triangle mask multiply on diagonal tiles.
    AV half A: lhsT = v2[:, i, 0:65] -> pav0[0:65] = [attn_A; den_A]
    AV half B: lhsT = v2[:, i, 64:129] -> pav1[63:128] = [den_B; attn_B]
      (attn_B lands directly on partitions 64:128 - no partition-shift DMA)
    normalize: reciprocal (DVE) + partition_broadcast (Pool) + multiply.
  P3 out-proj: py[t,e] = sum_ks attn^T[128ks, t] @ wo[128ks, e], psum ->
    fp16 staging (Pool) -> one DMA per 128-token row block.
  Scheduling: attention chunk qc runs right after proj chunk qc; out-proj
  matmuls for chunk qc-1 are emitted one-at-a-time between attention tiles
  as PE filler while ACT works through the exps.
"""

import os

import numpy as np

import concourse.mybir as mybir
import concourse.tile as tile
from concourse import bacc
from concourse import bass_utils

P = 128
B = 2
T = 2048
C = 2048
HD = 64
QH = 32
KVH = 8
NCORES = 8
TCH = 512   # token chunk for projection phase
QCH = 512   # q chunk for attention phase
KT = C // P  # 16 contraction tiles
NSUB = 6     # 4 Q pairs + K2 + V2
f32 = mybir.dt.float32
fp16 = mybir.dt.float16

_CACHE = {}


def _build():
    nc = bacc.Bacc("TRN2", target_bir_lowering=False, debug=False, num_devices=NCORES)

    xt = nc.dram_tensor("xt", [C, T], fp16, kind="ExternalInput")
    # sub-major, pre-rearranged on host: [sub, p, ko, m] so a per-sub load is
    # one 4KB-descriptor DMA
    wqkv = nc.dram_tensor("wqkv", [NSUB, P, KT, P], fp16, kind="ExternalInput")
    wo = nc.dram_tensor("wo", [4 * P, C], fp16, kind="ExternalInput")
    bqkv = nc.dram_tensor("bqkv", [P, NSUB], f32, kind="ExternalInput")
    maskd = nc.dram_tensor("mask", [P, P], fp16, kind="ExternalInput")
    identd = nc.dram_tensor("ident", [P, P], fp16, kind="ExternalInput")
    y = nc.dram_tensor("y", [T, C], fp16, kind="ExternalOutput")

    wo3 = wo.ap().rearrange("(ko p) m -> p ko m", p=P)
    xb = xt.ap().rearrange("(ko p) t -> p ko t", p=P)

    Exp = mybir.ActivationFunctionType.Exp
    mult = mybir.AluOpType.mult
    add = mybir.AluOpType.add

    with tile.TileContext(nc) as tc:
        with (
            tc.tile_pool(name="const", bufs=1) as cpool,
            tc.tile_pool(name="x", bufs=2) as xpool,
            tc.tile_pool(name="res", bufs=1) as apool,
            tc.tile_pool(name="vt", bufs=2) as vtpool,
            tc.tile_pool(name="expS", bufs=3) as wpool,
            tc.tile_pool(name="den", bufs=2) as dpool,
            tc.tile_pool(name="y", bufs=2) as ypool,
            tc.tile_pool(name="psA", bufs=2, space="PSUM") as psumA,
            tc.tile_pool(name="psB", bufs=2, space="PSUM") as psumB,
            tc.tile_pool(name="psC", bufs=2, space="PSUM") as psumC,
        ):
            # ---- constants / weights (resident) ----
            # startup-critical DMA order: wqkv sub0, x chunk 0 halves (the
            # first 16 proj matmuls need only these), then the rest.
            wqkv_sb = cpool.tile([P, NSUB, KT, P], fp16)
            xch0 = xpool.tile([P, KT, TCH], fp16, tag="xch", name="xch")
            # startup-critical order: sub0 weights, x chunk 0 halves, then
            # remaining subs one DMA each (4KB descriptors)
            nc.sync.dma_start(wqkv_sb[:, 0], wqkv.ap()[0])
            nc.sync.dma_start(xch0[:, 0 : KT // 2, :], xb[:, 0 : KT // 2, 0:TCH])
            nc.sync.dma_start(xch0[:, KT // 2 :, :], xb[:, KT // 2 :, 0:TCH])
            for _s in range(1, NSUB):
                nc.sync.dma_start(wqkv_sb[:, _s], wqkv.ap()[_s])
            bqkv_sb = cpool.tile([P, NSUB], f32)
            nc.sync.dma_start(bqkv_sb[:], bqkv.ap())
            mask_sb = cpool.tile([P, P], fp16)
            nc.sync.dma_start(mask_sb[:], maskd.ap())
            ident_sb = cpool.tile([P, P], fp16)
            nc.sync.dma_start(ident_sb[:], identd.ap())
            wo_sb = cpool.tile([P, 4, C], fp16)

            q_sb = apool.tile([P, 4, T], fp16, tag="q")
            k2_sb = apool.tile([P, T], fp16, tag="k2")
            # v2 cols: [V_g0 (0:64) | ones (64) | V_g1 (65:129) | ones (129)]
            v2_sb = apool.tile([P, KT, 130], fp16, tag="v2")
            attn_sb = apool.tile([P, 4, T], fp16, tag="attn")
            nc.gpsimd.memset(v2_sb[:, :, 64:65], 1.0)
            nc.gpsimd.memset(v2_sb[:, :, 129:130], 1.0)
            ones_f32 = cpool.tile([P, 1], f32)
            nc.gpsimd.memset(ones_f32[:], 1.0)
            ones_r = cpool.tile([P, HD], mybir.dt.float32r)
            nc.vector.tensor_copy(ones_r[:], ones_f32[:, 0:1].to_broadcast((P, HD)))

            # ---- filler queue: closures each emitting ~one PE matmul ----
            fillers = []

            def pop_filler(k=1):
                for _ in range(k):
                    if fillers:
                        fillers.pop(0)()

            def make_p3_fillers(qc):
                """Out-proj for token chunk qc: 4 row-blocks x 4 col-chunks,
                each a 4-matmul psum accumulation + Pool copy; one DMA per
                row-block."""
                out = []
                prev_dma = None
                for ts in range(qc * (QCH // P), (qc + 1) * (QCH // P)):
                    state = {}
                    mms = []
                    for ec in range(C // QCH):
                        for ks in range(4):
                            def mm(ts=ts, ec=ec, ks=ks, state=state):
                                if ks == 0 and ec == 0:
                                    state["y"] = ypool.tile(
                                        [P, C], fp16, tag="ysb", name="ysb"
                                    )
                                if ks == 0:
                                    state["py"] = psumC.tile(
                                        [P, QCH], f32, tag="pp", name="py"
                                    )
                                nc.tensor.matmul(
                                    state["py"][:],
                                    attn_sb[:, ks, ts * P : (ts + 1) * P],
                                    wo_sb[:, ks, ec * QCH : (ec + 1) * QCH],
                                    start=(ks == 0),
                                    stop=(ks == 3),
                                    skip_group_check=True,
                                )
                                if ks == 3:
                                    nc.vector.tensor_copy(
                                        state["y"][:, ec * QCH : (ec + 1) * QCH],
                                        state["py"][:],
                                    )
                            mms.append(mm)

                    def ydma(ts=ts, state=state):
                        nc.sync.dma_start(
                            y.ap()[ts * P : (ts + 1) * P, :], state["y"][:]
                        )

                    # defer each row-block's output DMA a few fillers past its
                    # last staging copy so its wait is satisfied when the SP
                    # queue reaches it
                    out.extend(mms[:4])
                    if prev_dma is not None:
                        out.append(prev_dma)
                    out.extend(mms[4:])
                    prev_dma = ydma
                out.append(prev_dma)
                return out

            # ---- P1: one projection chunk ----
            def proj_chunk(tch, xch):
                tsl = slice(tch * TCH, (tch + 1) * TCH)
                for sub in range(NSUB):
                    pp = psumC.tile([P, TCH], f32, tag="pp", name="pp")
                    for k in range(KT):
                        nc.tensor.matmul(
                            pp[:],
                            wqkv_sb[:, sub, k, :],
                            xch[:, k, :],
                            start=(k == 0),
                            stop=(k == KT - 1),
                            skip_group_check=True,
                        )
                    bias = bqkv_sb[:, sub : sub + 1].to_broadcast((P, TCH))
                    if sub < 4:
                        nc.vector.tensor_tensor(q_sb[:, sub, tsl], pp[:], bias, add)
                    elif sub == 4:
                        nc.vector.tensor_tensor(k2_sb[:, tsl], pp[:], bias, add)
                    else:
                        vt = vtpool.tile([P, TCH], fp16, tag="vt")
                        nc.vector.tensor_tensor(vt[:], pp[:], bias, add)
                        for ts in range(TCH // P):
                            tidx = tch * (TCH // P) + ts
                            pt = psumC.tile([P, P], fp16, tag="pp", name="pt")
                            nc.tensor.transpose(
                                pt[:], vt[:, ts * P : (ts + 1) * P], ident_sb[:]
                            )
                            nc.scalar.copy(v2_sb[:, tidx, 0:64], pt[:, 0:64])
                            nc.scalar.copy(v2_sb[:, tidx, 65:129], pt[:, 64:128])

            # ---- P2: attention for one (pair, q-chunk) ----
            def attn_pair(qc, pair):
                q0 = qc * QCH
                nfull = q0 // P
                ntiles = nfull + QCH // P
                pav0 = psumB.tile([P, QCH], f32, tag="pav", name="pav0")
                pav1 = psumB.tile([P, QCH], f32, tag="pav", name="pav1")
                for i in range(ntiles):
                    lo = 0 if i < nfull else (i - nfull) * P
                    nsl = slice(lo, QCH)
                    qsl = slice(q0 + lo, q0 + QCH)
                    ksl = slice(i * P, (i + 1) * P)
                    ps_s = psumA.tile([P, 2, QCH], f32, tag="ps", name="ps_s")
                    nc.tensor.matmul(
                        ps_s[:, 0, nsl],
                        k2_sb[0:64, ksl],
                        q_sb[0:64, pair, qsl],
                        start=True,
                        stop=True,
                        skip_group_check=True,
                    )
                    nc.tensor.matmul(
                        ps_s[:, 1, nsl],
                        k2_sb[64:128, ksl],
                        q_sb[64:128, pair, qsl],
                        start=True,
                        stop=True,
                        skip_group_check=True,
                    )
                    expS = wpool.tile([P, 2, QCH], fp16, tag="expS")
                    nc.scalar.activation(expS[:, :, nsl], ps_s[:, :, nsl], Exp, scale=0.125)
                    if i >= nfull:
                        j = i - nfull
                        jsl = slice(j * P, (j + 1) * P)
                        nc.vector.tensor_tensor(
                            expS[:, :, jsl],
                            expS[:, :, jsl],
                            mask_sb[:, None, :].to_broadcast((P, 2, P)),
                            mult,
                        )
                    nc.tensor.matmul(
                        pav0[0:65, nsl],
                        v2_sb[:, i, 0:65],
                        expS[:, 0, nsl],
                        start=(i == 0),
                        stop=(i == ntiles - 1),
                        skip_group_check=True,
                    )
                    nc.tensor.matmul(
                        pav1[0:65, nsl],
                        v2_sb[:, i, 65:130],
                        expS[:, 1, nsl],
                        start=(i == 0),
                        stop=(i == ntiles - 1),
                        skip_group_check=True,
                    )
                    pop_filler(1)
                # normalize: pav0/pav1 rows = [attn (0:64), den (64)]
                qsl = slice(q0, q0 + QCH)
                # den broadcast to 64 partitions via a K=1 ones matmul, then
                # reciprocal to SBUF so the normalize reads only one PSUM
                # operand
                den_sb = dpool.tile([P, 2, QCH], mybir.dt.float32r, tag="den")
                nc.vector.tensor_copy(den_sb[64:65, 0, :], pav0[64:65, :])
                nc.vector.tensor_copy(den_sb[64:65, 1, :], pav1[64:65, :])
                bc = psumA.tile([64, 2, QCH], f32, tag="ps", name="bc")
                for h in range(2):
                    nc.tensor.matmul(
                        bc[:, h, :],
                        ones_r[64:65, 0:64],
                        den_sb[64:65, h, :],
                        start=True,
                        stop=True,
                        skip_group_check=True,
                    )
                rec = dpool.tile([64, 2, QCH], f32, tag="rec")
                nc.vector.reciprocal(rec[:, 0, :], bc[:, 0, :])
                nc.vector.reciprocal(rec[:, 1, :], bc[:, 1, :])
                nc.vector.tensor_tensor(
                    attn_sb[0:64, pair, qsl], pav0[0:64, :], rec[:, 0, :], mult
                )
                alo = dpool.tile([64, QCH], fp16, tag="alo")
                nc.vector.tensor_tensor(alo[:], pav1[0:64, :], rec[:, 1, :], mult)
                nc.sync.dma_start(attn_sb[64:128, pair, qsl], alo[:])

            # ---- main schedule ----
            for tch in range(T // TCH):
                if tch == 0:
                    xch = xch0
                else:
                    xch = xpool.tile([P, KT, TCH], fp16, tag="xch", name="xch")
                    nc.sync.dma_start(
                        xch[:, 0 : KT // 2, :], xb[:, 0 : KT // 2, tch * TCH : (tch + 1) * TCH]
                    )
                    nc.sync.dma_start(
                        xch[:, KT // 2 :, :], xb[:, KT // 2 :, tch * TCH : (tch + 1) * TCH]
                    )
                proj_chunk(tch, xch)
                if tch == 0:
                    nc.sync.dma_start(wo_sb[:], wo3)
                for pair in range(4):
                    attn_pair(tch, pair)
                    pop_filler(3)
                fillers.extend(make_p3_fillers(tch))
            # drain remaining out-proj work (last chunk's + any leftovers)
            while fillers:
                pop_filler(1)

    nc.compile()
    return nc


def _prep_inputs(x, Wq, bq, Wk, bk, Wv, bv, Wo, bo):
    x = np.ascontiguousarray(np.asarray(x, dtype=np.float32))
    Wq = np.asarray(Wq, dtype=np.float32)
    Wk = np.asarray(Wk, dtype=np.float32)
    Wv = np.asarray(Wv, dtype=np.float32)
    Wo = np.asarray(Wo, dtype=np.float32)
    bq = np.asarray(bq, dtype=np.float32)
    bk = np.asarray(bk, dtype=np.float32)
    bv = np.asarray(bv, dtype=np.float32)

    xts = [np.ascontiguousarray(x[b].T).astype(np.float16) for b in range(B)]
    # mask[kj, qi] = 1 iff kj <= qi  (upper triangular incl. diag)
    mask = np.triu(np.ones((P, P), dtype=np.float16)).copy()
    ident = np.eye(P, dtype=np.float16)
    in_maps = []
    for c in range(NCORES):
        b, j = divmod(c, 4)
        # q heads: g0 = 8j..8j+3 (kv head 2j), g1 = 8j+4..8j+7 (kv 2j+1)
        qcols, wocols, bqc = [], [], []
        for i in range(4):
            for h in (8 * j + i, 8 * j + 4 + i):
                qcols.append(Wq[:, h * HD : (h + 1) * HD])
                wocols.append(Wo[h * HD : (h + 1) * HD, :])
                bqc.append(bq[h * HD : (h + 1) * HD])
        ks = slice(2 * j * HD, (2 * j + 2) * HD)
        wqkv_c = np.concatenate(qcols + [Wk[:, ks], Wv[:, ks]], axis=1)
        # [C, 768] -> [sub, p, ko, m] (sub-major, 4KB contiguous per (sub, p))
        wqkv_r = wqkv_c.reshape(KT, P, NSUB, P).transpose(2, 1, 0, 3)
        bqkv_c = np.stack(
            [np.concatenate(bqc[2 * i : 2 * i + 2]) for i in range(4)]
            + [bk[ks], bv[ks]],
            axis=1,
        )
        in_maps.append(
            {
                "xt": xts[b],
                "wqkv": np.ascontiguousarray(wqkv_r).astype(np.float16),
                "wo": np.ascontiguousarray(np.concatenate(wocols, axis=0)).astype(
                    np.float16
                ),
                "bqkv": np.ascontiguousarray(bqkv_c),
                "mask": mask,
                "ident": ident,
            }
        )
    return in_maps


def kernel(x, Wq, bq, Wk, bk, Wv, bv, Wo, bo, _trace=False):
    # NTFF tracing is unavailable through this axon client; make sure a
    # stray BASS_TRACE=1 in the environment cannot divert the run path.
    if not _trace:
        os.environ["BASS_NEVER_TRACE"] = "1"
    if "nc" not in _CACHE:
        _CACHE["nc"] = _build()
    nc = _CACHE["nc"]
    in_maps = _prep_inputs(x, Wq, bq, Wk, bk, Wv, bv, Wo, bo)
    res = bass_utils.run_bass_kernel_spmd(
        nc, in_maps, core_ids=list(range(NCORES)), trace=_trace
    )
    bo = np.asarray(bo, dtype=np.float32)
    y = np.zeros((B, T, C), dtype=np.float32)
    for c in range(NCORES):
        y[c // 4] += res.results[c]["y"].astype(np.float32)
    y += bo
    if _trace:
        return y, res
    return y


# revision 39
# speedup vs baseline: 1.4040x; 1.1530x over previous
"""GQA forward kernel for Trainium2, 8-core tensor-parallel (group-aligned).

Problem: B=2, T=2048, D=2048, 32 Q heads / 8 KV heads, head_dim 64, causal.

Sharding: core c = (batch b = c//4, kv-head pair j = c%4).  Each core owns
kv heads {2j, 2j+1} and their 8 q heads for ONE batch.  Each core reads only
its batch's x^T (fp16) and emits a row-parallel partial of the output
projection (fp16); the host sums 4 partials per batch (+ bo).

All matmuls in fp16 with fp32 PSUM accumulation (tolerance is 2e-2; fp16
keeps rel err ~1e-3 and always hits 1.0 cycles/row on the PE cost model).

Per-core dataflow:
  P1 (proj, 4 chunks of 512 tokens): lhsT = wqkv [C,768] fp16, rhs = x^T
    chunk -> 6 sub-blocks of 128: subs 0-3 = Q pairs [g0hi | g1hi],
    sub 4 = K2 = [K_g0 | K_g1]^T, sub 5 = V2^T (transposed to natural via
    PE identity-matmul transposes).  v2 layout [kv, 129] = [V_g0 | 1 | V_g1].
  P2 attention per (pair i, q-chunk of 512): scores transposed,
    S^T[kv, q] for both heads of the pair in one PSUM tile [128,2,512]
    (head g0hi contracts K2[0:64], g1hi contracts K2[64:128]).
    expS = ACT Exp(S/8) -> fp16; causal via column-sliced matmuls + one
    triangle mask multiply on diagonal tiles.
    AV half A: lhsT = v2[:, i, 0:65] -> pav0[0:65] = [attn_A; den_A]
    AV half B: lhsT = v2[:, i, 64:129] -> pav1[63:128] = [den_B; attn_B]
      (attn_B lands directly on partitions 64:128 - no partition-shift DMA)
    normalize: reciprocal (DVE) + partition_broadcast (Pool) + multiply.
  P3 out-proj: py[t,e] = sum_ks attn^T[128ks, t] @ wo[128ks, e], psum ->
    fp16 staging (Pool) -> one DMA per 128-token row block.
  Scheduling: attention chunk qc runs right after proj chunk qc; out-proj
  matmuls for chunk qc-1 are emitted one-at-a-time between attention tiles
  as PE filler while ACT works through the exps.
"""

import os

import numpy as np

import concourse.mybir as mybir
import concourse.tile as tile
from concourse import bacc
from concourse import bass_utils

P = 128
B = 2
T = 2048
C = 2048
HD = 64
QH = 32
KVH = 8
NCORES = 8
TCH = 512   # token chunk for projection phase
QCH = 512   # q chunk for attention phase
KT = C // P  # 16 contraction tiles
NSUB = 6     # 4 Q pairs + K2 + V2
f32 = mybir.dt.float32
fp16 = mybir.dt.float16

_CACHE = {}


def _build():
    nc = bacc.Bacc("TRN2", target_bir_lowering=False, debug=False, num_devices=NCORES)

    xt = nc.dram_tensor("xt", [C, T], fp16, kind="ExternalInput")
    # sub-major, pre-rearranged on host: [sub, p, ko, m] so a per-sub load is
    # one 4KB-descriptor DMA
    wqkv = nc.dram_tensor("wqkv", [NSUB, P, KT, P], fp16, kind="ExternalInput")
    wo = nc.dram_tensor("wo", [4 * P, C], fp16, kind="ExternalInput")
    bqkv = nc.dram_tensor("bqkv", [P, NSUB], f32, kind="ExternalInput")
    maskd = nc.dram_tensor("mask", [P, P], fp16, kind="ExternalInput")
    identd = nc.dram_tensor("ident", [P, P], fp16, kind="ExternalInput")
    y = nc.dram_tensor("y", [T, C], fp16, kind="ExternalOutput")

    wo3 = wo.ap().rearrange("(ko p) m -> p ko m", p=P)
    xb = xt.ap().rearrange("(ko p) t -> p ko t", p=P)

    Exp = mybir.ActivationFunctionType.Exp
    mult = mybir.AluOpType.mult
    add = mybir.AluOpType.add

    with tile.TileContext(nc) as tc:
        with (
            tc.tile_pool(name="const", bufs=1) as cpool,
            tc.tile_pool(name="x", bufs=2) as xpool,
            tc.tile_pool(name="res", bufs=1) as apool,
            tc.tile_pool(name="vt", bufs=2) as vtpool,
            tc.tile_pool(name="expS", bufs=16) as wpool,
            tc.tile_pool(name="den", bufs=2) as dpool,
            tc.tile_pool(name="y", bufs=2) as ypool,
            tc.tile_pool(name="psA", bufs=2, space="PSUM") as psumA,
            tc.tile_pool(name="psB", bufs=2, space="PSUM") as psumB,
            tc.tile_pool(name="psC", bufs=2, space="PSUM") as psumC,
        ):
            # ---- constants / weights (resident) ----
            # startup-critical DMA order: wqkv sub0, x chunk 0 halves (the
            # first 16 proj matmuls need only these), then the rest.
            wqkv_sb = cpool.tile([P, NSUB, KT, P], fp16)

            def xch_alloc(tch):
                # two tiles so matmuls on the first 8 K-slices need not wait
                # for the second half's DMA
                xlo = xpool.tile([P, KT // 2, TCH], fp16, tag="xlo", name="xlo")
                xhi = xpool.tile([P, KT // 2, TCH], fp16, tag="xhi", name="xhi")
                tsl = slice(tch * TCH, (tch + 1) * TCH)
                nc.sync.dma_start(xlo[:], xb[:, 0 : KT // 2, tsl])
                nc.sync.dma_start(xhi[:], xb[:, KT // 2 :, tsl])
                return (xlo, xhi)

            # startup-critical order: sub0 weights, x chunk 0 halves, then
            # remaining subs one DMA each (4KB descriptors)
            nc.sync.dma_start(wqkv_sb[:, 0], wqkv.ap()[0])
            xch0 = xch_alloc(0)
            for _s in range(1, NSUB):
                nc.sync.dma_start(wqkv_sb[:, _s], wqkv.ap()[_s])
            bqkv_sb = cpool.tile([P, NSUB], f32)
            nc.sync.dma_start(bqkv_sb[:], bqkv.ap())
            mask_sb = cpool.tile([P, P], fp16)
            nc.sync.dma_start(mask_sb[:], maskd.ap())
            ident_sb = cpool.tile([P, P], fp16)
            nc.sync.dma_start(ident_sb[:], identd.ap())
            wo_sb = cpool.tile([P, 4, C], fp16)

            q_sb = apool.tile([P, 4, T], fp16, tag="q")
            k2_sb = apool.tile([P, T], fp16, tag="k2")
            # v2 cols: [V_g0 (0:64) | ones (64) | V_g1 (65:129) | ones (129)]
            v2_sb = apool.tile([P, KT, 130], fp16, tag="v2")
            attn_sb = apool.tile([P, 4, T], fp16, tag="attn")
            nc.gpsimd.memset(v2_sb[:, :, 64:65], 1.0)
            nc.gpsimd.memset(v2_sb[:, :, 129:130], 1.0)
            ones_f32 = cpool.tile([P, 1], f32)
            nc.gpsimd.memset(ones_f32[:], 1.0)
            ones_r = cpool.tile([P, HD], mybir.dt.float32r)
            nc.vector.tensor_copy(ones_r[:], ones_f32[:, 0:1].to_broadcast((P, HD)))

            # ---- filler queues: closures each emitting ~one PE matmul.
            # proj fillers have a deadline (their attention chunk) and pop
            # first; p3 fillers drain opportunistically.
            fillers_proj = []
            fillers_p3 = []

            def pop_filler(k=1):
                for _ in range(k):
                    if fillers_proj:
                        fillers_proj.pop(0)()
                    elif fillers_p3:
                        fillers_p3.pop(0)()

            def make_p3_fillers(qc):
                """Out-proj for token chunk qc: 4 row-blocks x 4 col-chunks,
                each a 4-matmul psum accumulation + Pool copy; one DMA per
                row-block."""
                out = []
                prev_dma = None
                for ts in range(qc * (QCH // P), (qc + 1) * (QCH // P)):
                    state = {}
                    mms = []
                    for ec in range(C // QCH):
                        for ks in range(4):
                            def mm(ts=ts, ec=ec, ks=ks, state=state):
                                if ks == 0 and ec == 0:
                                    state["y"] = ypool.tile(
                                        [P, C], fp16, tag="ysb", name="ysb"
                                    )
                                if ks == 0:
                                    state["py"] = psumC.tile(
                                        [P, QCH], f32, tag="pp", name="py"
                                    )
                                nc.tensor.matmul(
                                    state["py"][:],
                                    attn_sb[:, ks, ts * P : (ts + 1) * P],
                                    wo_sb[:, ks, ec * QCH : (ec + 1) * QCH],
                                    start=(ks == 0),
                                    stop=(ks == 3),
                                    skip_group_check=True,
                                )
                                if ks == 3:
                                    nc.vector.tensor_copy(
                                        state["y"][:, ec * QCH : (ec + 1) * QCH],
                                        state["py"][:],
                                    )
                            mms.append(mm)

                    def ydma(ts=ts, state=state):
                        nc.sync.dma_start(
                            y.ap()[ts * P : (ts + 1) * P, :], state["y"][:]
                        )

                    # defer each row-block's output DMA a few fillers past its
                    # last staging copy so its wait is satisfied when the SP
                    # queue reaches it
                    out.extend(mms[:4])
                    if prev_dma is not None:
                        out.append(prev_dma)
                    out.extend(mms[4:])
                    prev_dma = ydma
                out.append(prev_dma)
                return out

            # ---- P1: one projection chunk, as a list of filler closures ----
            def proj_fillers(tch, xch):
                tsl = slice(tch * TCH, (tch + 1) * TCH)
                out = []
                for sub in range(NSUB):
                    state = {}
                    for k in range(KT):
                        def mm(sub=sub, k=k, state=state):
                            if k == 0:
                                state["pp"] = psumC.tile(
                                    [P, TCH], f32, tag="pp", name="pp"
                                )
                            nc.tensor.matmul(
                                state["pp"][:],
                                wqkv_sb[:, sub, k, :],
                                xch[k // (KT // 2)][:, k % (KT // 2), :],
                                start=(k == 0),
                                stop=(k == KT - 1),
                                skip_group_check=True,
                            )
                            if k == KT - 1:
                                bias = bqkv_sb[:, sub : sub + 1].to_broadcast(
                                    (P, TCH)
                                )
                                if sub < 4:
                                    nc.vector.tensor_tensor(
                                        q_sb[:, sub, tsl], state["pp"][:], bias, add
                                    )
                                elif sub == 4:
                                    nc.vector.tensor_tensor(
                                        k2_sb[:, tsl], state["pp"][:], bias, add
                                    )
                                else:
                                    state["vt"] = vtpool.tile(
                                        [P, TCH], fp16, tag="vt", name="vt"
                                    )
                                    nc.vector.tensor_tensor(
                                        state["vt"][:], state["pp"][:], bias, add
                                    )
                        out.append(mm)
                    if sub == 5:
                        for ts in range(TCH // P):
                            def tpose(ts=ts, tch=tch, state=state):
                                tidx = tch * (TCH // P) + ts
                                pt = psumC.tile([P, P], fp16, tag="pp", name="pt")
                                nc.tensor.transpose(
                                    pt[:],
                                    state["vt"][:, ts * P : (ts + 1) * P],
                                    ident_sb[:],
                                )
                                nc.vector.tensor_copy(v2_sb[:, tidx, 0:64], pt[:, 0:64])
                                nc.vector.tensor_copy(
                                    v2_sb[:, tidx, 65:129], pt[:, 64:128]
                                )
                            out.append(tpose)
                return out

            def proj_chunk(tch, xch):
                for fn in proj_fillers(tch, xch):
                    fn()

            # ---- P2: attention for one (pair, q-chunk) ----
            # AV is "flipped": expS tiles are the PE stationary operand and V
            # streams through (65 cols instead of up to 512), halving AV
            # engine time.  Output pq[q, d] has the denominator at column 64
            # of each region, so normalize is a per-partition scalar multiply;
            # a PE transpose restores attn^T for the output projection.
            def attn_pair(qc, pair):
                q0 = qc * QCH
                nfull = q0 // P
                ntiles = nfull + QCH // P
                exps = []
                for i in range(ntiles):
                    lo = 0 if i < nfull else (i - nfull) * P
                    nsl = slice(lo, QCH)
                    qsl = slice(q0 + lo, q0 + QCH)
                    ksl = slice(i * P, (i + 1) * P)
                    ps_s = psumA.tile([P, 2, QCH], f32, tag="ps", name="ps_s")
                    nc.tensor.matmul(
                        ps_s[:, 0, nsl],
                        k2_sb[0:64, ksl],
                        q_sb[0:64, pair, qsl],
                        start=True,
                        stop=True,
                        skip_group_check=True,
                    )
                    nc.tensor.matmul(
                        ps_s[:, 1, nsl],
                        k2_sb[64:128, ksl],
                        q_sb[64:128, pair, qsl],
                        start=True,
                        stop=True,
                        skip_group_check=True,
                    )
                    expS = wpool.tile([P, 2, QCH], fp16, tag="expS")
                    nc.scalar.activation(expS[:, :, nsl], ps_s[:, :, nsl], Exp, scale=0.125)
                    if i >= nfull:
                        j = i - nfull
                        jsl = slice(j * P, (j + 1) * P)
                        nc.vector.tensor_tensor(
                            expS[:, :, jsl],
                            expS[:, :, jsl],
                            mask_sb[:, None, :].to_broadcast((P, 2, P)),
                            mult,
                        )
                    exps.append(expS)
                    pop_filler(2)
                # AV per [128q, 65] region: one sequential accumulation group
                # per PSUM bank; den lands in column 64, so normalize is a
                # per-partition scalar multiply; a PE transpose restores
                # attn^T for the output projection.
                qsl = slice(q0, q0 + QCH)
                tp = psumC.tile([P, 4, P], fp16, tag="pp", name="tp")
                for h in range(2):
                    rec = dpool.tile([P, 4, 1], f32, tag="rec")
                    attn_n = dpool.tile([P, 4, HD], fp16, tag="attn_n")
                    for jj in range(QCH // P):
                        pq = psumB.tile([P, 65], f32, tag="pq", name="pq")
                        last = nfull + jj
                        for i in range(last + 1):
                            nc.tensor.matmul(
                                pq[:],
                                exps[i][:, h, jj * P : (jj + 1) * P],
                                v2_sb[:, i, h * 65 : h * 65 + 65],
                                start=(i == 0),
                                stop=(i == last),
                                skip_group_check=True,
                            )
                        nc.vector.reciprocal(rec[:, jj, :], pq[:, 64:65])
                        nc.vector.tensor_tensor(
                            attn_n[:, jj, :],
                            pq[:, 0:64],
                            rec[:, jj, :].to_broadcast((P, HD)),
                            mult,
                        )
                        nc.tensor.transpose(
                            tp[h * HD : (h + 1) * HD, jj, :],
                            attn_n[:, jj, :],
                            ident_sb[:],
                        )
                        pop_filler(1)
                nc.vector.tensor_copy(attn_sb[:, pair, qsl], tp[:])

            # ---- main schedule ----
            # proj(0) runs as a block; proj(qc+1) and out-proj(qc-1) pop as
            # fillers between attention tiles of chunk qc, keeping the PE fed
            # while ACT works through the exps.
            proj_chunk(0, xch0)
            nc.sync.dma_start(wo_sb[:], wo3)
            for tch in range(T // TCH):
                if tch < T // TCH - 1:
                    fillers_proj.extend(proj_fillers(tch + 1, xch_alloc(tch + 1)))
                for pair in range(4):
                    attn_pair(tch, pair)
                    pop_filler(2)
                # the next chunk's attention needs its projections complete
                while fillers_proj:
                    pop_filler(1)
                fillers_p3.extend(make_p3_fillers(tch))
            while fillers_p3:
                pop_filler(1)

    nc.compile()
    return nc


def _prep_inputs(x, Wq, bq, Wk, bk, Wv, bv, Wo, bo):
    x = np.ascontiguousarray(np.asarray(x, dtype=np.float32))
    Wq = np.asarray(Wq, dtype=np.float32)
    Wk = np.asarray(Wk, dtype=np.float32)
    Wv = np.asarray(Wv, dtype=np.float32)
    Wo = np.asarray(Wo, dtype=np.float32)
    bq = np.asarray(bq, dtype=np.float32)
    bk = np.asarray(bk, dtype=np.float32)
    bv = np.asarray(bv, dtype=np.float32)

    xts = [np.ascontiguousarray(x[b].T).astype(np.float16) for b in range(B)]
    # mask[kj, qi] = 1 iff kj <= qi  (upper triangular incl. diag)
    mask = np.triu(np.ones((P, P), dtype=np.float16)).copy()
    ident = np.eye(P, dtype=np.float16)
    in_maps = []
    for c in range(NCORES):
        b, j = divmod(c, 4)
        # q heads: g0 = 8j..8j+3 (kv head 2j), g1 = 8j+4..8j+7 (kv 2j+1)
        qcols, wocols, bqc = [], [], []
        for i in range(4):
            for h in (8 * j + i, 8 * j + 4 + i):
                qcols.append(Wq[:, h * HD : (h + 1) * HD])
                wocols.append(Wo[h * HD : (h + 1) * HD, :])
                bqc.append(bq[h * HD : (h + 1) * HD])
        ks = slice(2 * j * HD, (2 * j + 2) * HD)
        wqkv_c = np.concatenate(qcols + [Wk[:, ks], Wv[:, ks]], axis=1)
        # [C, 768] -> [sub, p, ko, m] (sub-major, 4KB contiguous per (sub, p))
        wqkv_r = wqkv_c.reshape(KT, P, NSUB, P).transpose(2, 1, 0, 3)
        bqkv_c = np.stack(
            [np.concatenate(bqc[2 * i : 2 * i + 2]) for i in range(4)]
            + [bk[ks], bv[ks]],
            axis=1,
        )
        in_maps.append(
            {
                "xt": xts[b],
                "wqkv": np.ascontiguousarray(wqkv_r).astype(np.float16),
                "wo": np.ascontiguousarray(np.concatenate(wocols, axis=0)).astype(
                    np.float16
                ),
                "bqkv": np.ascontiguousarray(bqkv_c),
                "mask": mask,
                "ident": ident,
            }
        )
    return in_maps


def kernel(x, Wq, bq, Wk, bk, Wv, bv, Wo, bo, _trace=False):
    # NTFF tracing is unavailable through this axon client; make sure a
    # stray BASS_TRACE=1 in the environment cannot divert the run path.
    if not _trace:
        os.environ["BASS_NEVER_TRACE"] = "1"
    if "nc" not in _CACHE:
        _CACHE["nc"] = _build()
    nc = _CACHE["nc"]
    in_maps = _prep_inputs(x, Wq, bq, Wk, bk, Wv, bv, Wo, bo)
    res = bass_utils.run_bass_kernel_spmd(
        nc, in_maps, core_ids=list(range(NCORES)), trace=_trace
    )
    bo = np.asarray(bo, dtype=np.float32)
    y = np.zeros((B, T, C), dtype=np.float32)
    for c in range(NCORES):
        y[c // 4] += res.results[c]["y"].astype(np.float32)
    y += bo
    if _trace:
        return y, res
    return y
